# revision 1
# baseline (speedup 1.0000x reference)
"""Causal multi-head attention on 8 trn2 NeuronCores.

Problem: B=4, S=2048, D=2048, H=16 heads, head_dim=128, causal softmax,
torch-style Linear projections (W stored [in, out]).

Sharding: core c handles batch b = c//2 and head-group g = c%2
(8 heads = 1024 output columns of Wq/Wk/Wv, 1024 rows of Wo).
Each core produces a partial output [S, D]; host sums the two
head-group partials per batch and adds bo.

Per-core device pipeline (all matmuls fp32r, 1 cycle/row):
  Phase A: from xT (host-pretransposed [D, S]) compute
           Q^T, K^T [1024, S] and V [S, 1024]; spill to DRAM scratch.
  Phase B: per head h: scores^T tiles [128 k, 512 q] = K_h Q_h^T,
           causal mask (additive, precomputed), exp (no max-subtract:
           scores are O(5), fp32 exp is safe), ctx^T accumulation
           C^T = V_h^T-blocks @ P^T, denominators via ones-matmul,
           normalize with reciprocal broadcast (PE outer product).
  Phase C: out_partial = C @ Wo_slice via C^T blocks as lhsT.
"""

import numpy as np

import concourse.bass as bass
import concourse.mybir as mybir
import concourse.tile as tile
from concourse import bacc
from concourse.bass_utils import run_bass_kernel_spmd

B = 4
S = 2048
D = 2048
H = 16
DH = 128
HPC = 8          # heads per core
DHG = HPC * DH   # 1024: head-group width per core
KT = D // 128    # 16 k-tiles over the model dim
ST = S // 128    # 16 s-tiles
QC = S // 512    # 4 q-chunks
SCALE = 1.0 / np.sqrt(DH)
NEG = -1.0e30

F32 = mybir.dt.float32
F32R = mybir.dt.float32r


def _build_nc():
    nc = bacc.Bacc(None, target_bir_lowering=False)

    xT = nc.declare_dram_parameter("xT", [D, S], F32, isOutput=False)
    # wq/wk host-pregathered to [HPC*128, KT*128]: row t*128+p, col n*128+m
    # = Wq[n*128+p, t*128+m] so each head-tile's weights DMA contiguously
    wq = nc.declare_dram_parameter("wq", [DHG, D], F32, isOutput=False)
    wk = nc.declare_dram_parameter("wk", [DHG, D], F32, isOutput=False)
    wv = nc.declare_dram_parameter("wv", [D, DHG], F32, isOutput=False)
    wo = nc.declare_dram_parameter("wo", [DHG, D], F32, isOutput=False)
    bqT = nc.declare_dram_parameter("bqT", [128, HPC], F32, isOutput=False)
    bkT = nc.declare_dram_parameter("bkT", [128, HPC], F32, isOutput=False)
    bvb = nc.declare_dram_parameter("bvb", [128, DHG], F32, isOutput=False)
    cmask = nc.declare_dram_parameter("cmask", [128, 896], F32, isOutput=False)
    out = nc.declare_dram_parameter("out", [S, D], F32, isOutput=True)

    with tile.TileContext(nc) as tc:
        _emit(nc, tc, xT, wq, wk, wv, wo, bqT, bkT, bvb, cmask, out)
    nc.compile()
    return nc


def _emit(nc, tc, xT, wq, wk, wv, wo, bqT, bkT, bvb, cmask, out):
    with (
        tc.tile_pool(name="const", bufs=1) as const,
        tc.tile_pool(name="dram", bufs=1, space="DRAM") as dram,
        tc.tile_pool(name="qkpre", bufs=2) as qkpre,
    ):
        qt_d = dram.tile([DHG, S], F32R)
        kt_d = dram.tile([DHG, S], F32R)
        v_d = dram.tile([S, DHG], F32R)
        ct_d = dram.tile([DHG, S], F32R)

        cm_sb = const.tile([128, 896], F32)
        nc.sync.dma_start(out=cm_sb, in_=cmask[:, :])
        bq_sb = const.tile([128, HPC], F32)
        nc.sync.dma_start(out=bq_sb, in_=bqT[:, :])
        bk_sb = const.tile([128, HPC], F32)
        nc.sync.dma_start(out=bk_sb, in_=bkT[:, :])
        bv_sb = const.tile([128, DHG], F32)
        nc.sync.dma_start(out=bv_sb, in_=bvb[:, :])
        ones_f32 = const.tile([128, 128], F32)
        nc.vector.memset(ones_f32, 1.0)
        ones128 = const.tile([128, 128], F32R)
        nc.vector.tensor_copy(out=ones128, in_=ones_f32)

        v_r = v_d[:, :].rearrange("(n p) d -> p n d", p=128)
        ct_r = ct_d[:, :].rearrange("(n p) m -> p n m", p=128)
        head0 = {}   # prefetched head-0 tiles, loaded during phase A
        ct_pre = {}  # prefetched phase-C ct tiles, loaded during phase B

        # ---------------- Phase A: projections, spilled to DRAM -------------
        wv_r = wv.bitcast(F32R).rearrange("(n p) m -> p n m", p=128)

        with (
            tc.tile_pool(name="xts", bufs=3) as xtp,
            tc.tile_pool(name="wqk", bufs=2) as wqk,
            tc.tile_pool(name="wvp", bufs=2) as wvp,
            tc.tile_pool(name="apsum", bufs=8, space="PSUM") as aps,
            tc.tile_pool(name="astage", bufs=4) as ast,
        ):
            for sh in range(2):
                s0 = sh * (S // 2)
                # weight tiles prefetched (depth 2) ahead of the bulk xT DMAs
                seq = [(w, b, dst, t)
                       for w, b, dst in ((wq, bq_sb, qt_d), (wk, bk_sb, kt_d))
                       for t in range(HPC)]
                w_tiles = {}

                def w_prefetch(i):
                    if i < len(seq):
                        w, _, _, t = seq[i]
                        w_sb = wqk.tile([128, KT, 128], F32R, tag="wqk",
                                        name=f"w_sb{i % 2}")
                        nc.sync.dma_start(
                            out=w_sb,
                            in_=w[t * 128 : (t + 1) * 128, :]
                            .rearrange("p (n m) -> p n m", m=128)
                            .bitcast(F32R),
                        )
                        w_tiles[i] = w_sb

                w_prefetch(0)

                # xT half as two sub-tiles of 8 k-tiles each (bufs=3 lets the
                # next half's first sub-tile prefetch during this half).
                xt_lo = xtp.tile([128, 8, S // 2], F32R, tag="xts")
                xt_hi = xtp.tile([128, 8, S // 2], F32R, tag="xts")

                def xt_blk(kd):
                    t = xt_lo if kd < 8 else xt_hi
                    return t[:, kd % 8, :]

                for kd in range(KT):
                    nc.sync.dma_start(
                        out=xt_blk(kd),
                        in_=xT[kd * 128 : (kd + 1) * 128, s0 : s0 + S // 2].bitcast(F32R),
                    )

                # Q^T and K^T: psum[dh 128, s 512] = sum_kd Wblk^T @ xTblk
                for i, (w, b_sb, dst, t) in enumerate(seq):
                    w_sb = w_tiles.pop(i)
                    w_prefetch(i + 1)
                    for sc in range(2):
                        psum = aps.tile([128, 512], F32, tag="apsum", name="qk_ps")
                        for kd in range(KT):
                            nc.tensor.matmul(
                                psum,
                                w_sb[:, kd, :],
                                xt_blk(kd)[:, sc * 512 : (sc + 1) * 512],
                                start=(kd == 0),
                                stop=(kd == KT - 1),
                            )
                        stg = ast.tile([128, 512], F32R, tag="astage")
                        nc.vector.tensor_scalar_add(
                            out=stg, in0=psum, scalar1=b_sb[:, t : t + 1]
                        )
                        nc.gpsimd.dma_start(
                            out=dst[
                                t * 128 : (t + 1) * 128,
                                s0 + sc * 512 : s0 + (sc + 1) * 512,
                            ],
                            in_=stg,
                        )

                if sh == 1:
                    qt0 = qkpre.tile([128, S], F32R, tag="qt", name="qt0")
                    nc.sync.dma_start(out=qt0, in_=qt_d[0:128, :])
                    kt0 = qkpre.tile([128, S], F32R, tag="kt", name="kt0")
                    nc.sync.dma_start(out=kt0, in_=kt_d[0:128, :])
                    head0["qt"] = qt0
                    head0["kt"] = kt0

                # V: psum[s 128, dh 512] = sum_kd xTblk^T @ Wvblk.
                # wv streamed two k-tiles per DMA; 4 s-tiles accumulate at once.
                for t2 in range(2):
                    # wv half-chunks (8 k-tiles each), double-buffered; all 8
                    # s-tiles of this half accumulate in one kd sweep
                    wv_lo = wvp.tile([128, 8, 512], F32R, tag="wvp", name="wv_lo")
                    nc.sync.dma_start(
                        out=wv_lo, in_=wv_r[:, 0:8, t2 * 512 : (t2 + 1) * 512]
                    )
                    wv_hi = wvp.tile([128, 8, 512], F32R, tag="wvp", name="wv_hi")
                    nc.sync.dma_start(
                        out=wv_hi, in_=wv_r[:, 8:16, t2 * 512 : (t2 + 1) * 512]
                    )
                    psums = [
                        aps.tile([128, 512], F32, tag="apsum", name=f"vps{si}")
                        for si in range(8)
                    ]
                    for kd in range(KT):
                        wv_blk = wv_lo if kd < 8 else wv_hi
                        for si in range(8):
                            nc.tensor.matmul(
                                psums[si],
                                xt_blk(kd)[:, si * 128 : (si + 1) * 128],
                                wv_blk[:, kd % 8, :],
                                start=(kd == 0),
                                stop=(kd == KT - 1),
                            )
                    for si in range(8):
                        stg = ast.tile([128, 512], F32R, tag="astage")
                        nc.vector.tensor_tensor(
                            out=stg,
                            in0=psums[si],
                            in1=bv_sb[:, t2 * 512 : (t2 + 1) * 512],
                            op=mybir.AluOpType.add,
                        )
                        nc.gpsimd.dma_start(
                            out=v_d[
                                s0 + si * 128 : s0 + (si + 1) * 128,
                                t2 * 512 : (t2 + 1) * 512,
                            ],
                            in_=stg,
                        )

        # ---------------- Phase B: per-head attention ------------------------
        wo_r = wo.bitcast(F32R).rearrange("(n p) m -> p n m", p=128)
        wop_cm = tc.tile_pool(name="wop", bufs=1)
        wop = wop_cm.__enter__()
        ctin_cm = tc.tile_pool(name="ctin", bufs=3)
        ctin = ctin_cm.__enter__()
        with (
            tc.tile_pool(name="vpool", bufs=2) as vpool,
            tc.tile_pool(name="ct", bufs=4) as ctpool,
            tc.tile_pool(name="ptile", bufs=6) as ppool,
            tc.tile_pool(name="msk", bufs=3) as mpool,
            tc.tile_pool(name="rcp", bufs=2) as rcpool,
            tc.tile_pool(name="pscore", bufs=3, space="PSUM") as pscore,
            tc.tile_pool(name="pctx", bufs=2, space="PSUM") as pctx,
            tc.tile_pool(name="psum2", bufs=2, space="PSUM") as psums,
        ):
            wo_sb = wop.tile([128, HPC, D], F32R)
            for h in range(HPC):
                if h == 0:
                    qt_sb = head0["qt"]
                    kt_sb = head0["kt"]
                else:
                    qt_sb = qkpre.tile([128, S], F32R, tag="qt", name="qt_sb")
                    nc.sync.dma_start(out=qt_sb, in_=qt_d[h * 128 : (h + 1) * 128, :])
                    kt_sb = qkpre.tile([128, S], F32R, tag="kt", name="kt_sb")
                    nc.sync.dma_start(out=kt_sb, in_=kt_d[h * 128 : (h + 1) * 128, :])
                v_sb = vpool.tile([128, ST, 128], F32R, tag="v", name="v_sb")
                nc.sync.dma_start(out=v_sb, in_=v_r[:, :, h * 128 : (h + 1) * 128])
                # spread the 8MB Wo load through phase B on the idle sync queue
                nc.sync.dma_start(out=wo_sb[:, h, :], in_=wo_r[:, h, :])

                for qc in range(QC):
                    nkt = 4 * qc + 4
                    # diagonal tiles first: their longer PE->DVE->ACT chains
                    # start early and overlap with the full tiles' stream
                    order = list(range(4 * qc, nkt)) + list(range(4 * qc))
                    psum_c = pctx.tile([128, 512], F32)
                    psum_s = psums.tile([128, 512], F32)

                    def scores(kt_i):
                        # diagonal tile j has valid columns only at qq >= 128j:
                        # compute just that [128, 512-128j] strip
                        j = kt_i - 4 * qc
                        off = 128 * j if j > 0 else 0
                        ps_t = pscore.tile([128, 512], F32, tag="ps_t")
                        nc.tensor.matmul(
                            ps_t[:, off:],
                            kt_sb[:, kt_i * 128 : (kt_i + 1) * 128],
                            qt_sb[:, qc * 512 + off : (qc + 1) * 512],
                            start=True,
                            stop=True,
                        )
                        p_t = ppool.tile([128, 512], F32R, tag="p_t")
                        if j >= 0:
                            msk = mpool.tile([128, 512], F32, tag="msk")
                            nc.vector.tensor_tensor(
                                out=msk[:, off:],
                                in0=ps_t[:, off:],
                                in1=cm_sb[:, 384 : 896 - off],
                                op=mybir.AluOpType.add,
                            )
                            src = msk
                        else:
                            src = ps_t
                        nc.scalar.activation(
                            out=p_t[:, off:],
                            in_=src[:, off:],
                            func=mybir.ActivationFunctionType.Exp,
                            scale=float(SCALE),
                        )
                        return p_t, off

                    def ctx(idx, kt_i, p_t, off):
                        nc.tensor.matmul(
                            psum_c[:, off:],
                            v_sb[:, kt_i, :],
                            p_t[:, off:],
                            start=(idx == 0),
                            stop=(idx == nkt - 1),
                        )
                        # every psum_s row accumulates the per-q denominator
                        nc.tensor.matmul(
                            psum_s[:, off:],
                            ones128,
                            p_t[:, off:],
                            start=(idx == 0),
                            stop=(idx == nkt - 1),
                        )

                    # software-pipeline scores/exp ahead of ctx by one tile
                    prev = None
                    for idx, kt_i in enumerate(order):
                        p_t, off = scores(kt_i)
                        if prev is not None:
                            ctx(idx - 1, prev[0], prev[1], prev[2])
                        prev = (kt_i, p_t, off)
                    ctx(nkt - 1, prev[0], prev[1], prev[2])

                    recip = rcpool.tile([128, 512], F32, tag="rcp")
                    nc.vector.reciprocal_approx_fast(out=recip, in_=psum_s)
                    ct = ctpool.tile([128, 512], F32R, tag="ct")
                    nc.vector.tensor_tensor(
                        out=ct,
                        in0=psum_c,
                        in1=recip,
                        op=mybir.AluOpType.mult,
                    )
                    nc.gpsimd.dma_start(
                        out=ct_d[h * 128 : (h + 1) * 128, qc * 512 : (qc + 1) * 512],
                        in_=ct,
                    )
                    if h == HPC - 1 and qc < 2:
                        pre = ctin.tile([128, HPC, 128], F32R, tag="ctin",
                                        name=f"ctpre{qc}")
                        nc.sync.dma_start(
                            out=pre,
                            in_=ct_r[:, :, qc * 4 * 128 : (qc * 4 + 1) * 128],
                        )
                        ct_pre[qc * 4] = pre

        # ---------------- Phase C: output projection -------------------------
        with (
            tc.tile_pool(name="opsum", bufs=4, space="PSUM") as ops,
            tc.tile_pool(name="ostage", bufs=4) as ost,
        ):
            for st in range(ST):
                if st in ct_pre:
                    ct_sb = ct_pre[st]
                else:
                    ct_sb = ctin.tile([128, HPC, 128], F32R, tag="ctin",
                                      name="ct_sb")
                    nc.sync.dma_start(
                        out=ct_sb, in_=ct_r[:, :, st * 128 : (st + 1) * 128]
                    )
                for ncol in range(4):
                    psum = ops.tile([128, 512], F32)
                    for hh in range(HPC):
                        nc.tensor.matmul(
                            psum,
                            ct_sb[:, hh, :],
                            wo_sb[:, hh, ncol * 512 : (ncol + 1) * 512],
                            start=(hh == 0),
                            stop=(hh == HPC - 1),
                        )
                    o_sb = ost.tile([128, 512], F32, tag="ostage")
                    nc.scalar.activation(
                        out=o_sb, in_=psum, func=mybir.ActivationFunctionType.Copy
                    )
                    nc.gpsimd.dma_start(
                        out=out[
                            st * 128 : (st + 1) * 128,
                            ncol * 512 : (ncol + 1) * 512,
                        ],
                        in_=o_sb,
                    )
        ctin_cm.__exit__(None, None, None)
        wop_cm.__exit__(None, None, None)


ctpool_tiles = {}

_NC = None


def _get_nc():
    global _NC
    if _NC is None:
        ctpool_tiles.clear()
        _NC = _build_nc()
    return _NC


def _host_prep(input_sequences, Wq, bq, Wk, bk, Wv, bv, Wo, bo):
    """Build per-core input maps."""
    x = np.asarray(input_sequences, dtype=np.float32)
    cm = np.full((128, 896), NEG, dtype=np.float32)
    kk = np.arange(128)[:, None]
    uu = np.arange(896)[None, :]
    cm[kk <= uu - 384] = 0.0

    in_maps = []
    for c in range(8):
        b, g = divmod(c, 2)
        sl = slice(g * DHG, (g + 1) * DHG)
        wq_c = np.ascontiguousarray(
            np.asarray(Wq[:, sl], dtype=np.float32)
            .reshape(KT, 128, HPC, 128).transpose(2, 1, 0, 3).reshape(DHG, D)
        )
        wk_c = np.ascontiguousarray(
            np.asarray(Wk[:, sl], dtype=np.float32)
            .reshape(KT, 128, HPC, 128).transpose(2, 1, 0, 3).reshape(DHG, D)
        )
        wv_c = np.ascontiguousarray(Wv[:, sl], dtype=np.float32)
        wo_c = np.ascontiguousarray(Wo[sl, :], dtype=np.float32)
        in_maps.append({
            "xT": np.ascontiguousarray(x[b].T),
            "wq": wq_c,
            "wk": wk_c,
            "wv": wv_c,
            "wo": wo_c,
            "bqT": np.ascontiguousarray(
                np.asarray(bq[sl], dtype=np.float32).reshape(HPC, 128).T
            ),
            "bkT": np.ascontiguousarray(
                np.asarray(bk[sl], dtype=np.float32).reshape(HPC, 128).T
            ),
            "bvb": np.ascontiguousarray(
                np.broadcast_to(np.asarray(bv[sl], dtype=np.float32), (128, DHG))
            ),
            "cmask": cm,
        })
    return in_maps


def kernel(input_sequences, Wq, bq, Wk, bk, Wv, bv, Wo, bo, _trace=False):
    nc = _get_nc()
    in_maps = _host_prep(input_sequences, Wq, bq, Wk, bk, Wv, bv, Wo, bo)
    res = run_bass_kernel_spmd(nc, in_maps, list(range(8)), trace=_trace)
    bo32 = np.asarray(bo, dtype=np.float32)
    out = np.empty((B, S, D), dtype=np.float32)
    for b in range(B):
        out[b] = res.results[2 * b]["out"] + res.results[2 * b + 1]["out"] + bo32
    if _trace:
        kernel.last_exec_time_ns = res.exec_time_ns
    return out



# revision 2
# speedup vs baseline: 1.2816x; 1.2816x over previous
"""Causal multi-head attention on 8 trn2 NeuronCores.

Problem: B=4, S=2048, D=2048, H=16 heads, head_dim=128, causal softmax,
torch-style Linear projections (W stored [in, out]).

Sharding: core c handles batch b = c//2 and head-group g = c%2
(8 heads = 1024 output columns of Wq/Wk/Wv, 1024 rows of Wo).
Each core produces a partial output [S, D]; host sums the two
head-group partials per batch and adds bo.

All matmul operands fp16 (PSUM accumulation stays fp32): fp16 streams
at 1 row/cycle on the PE vs fp32r's ~1.25, and halves DMA/SBUF bytes.

Per-core device pipeline:
  Phase A: from xT (host-pretransposed [D, S]) compute
           Q^T, K^T [1024, S] and V [S, 1024]; spill to DRAM scratch.
  Phase B: per head h: scores^T tiles [128 k, 512 q] = K_h Q_h^T,
           causal mask (additive, precomputed), exp (no max-subtract:
           scores are O(5), exp is safe), ctx^T accumulation
           C^T = V_h^T-blocks @ P^T, denominators via ones-matmul,
           normalize with reciprocal broadcast (PE outer product).
  Phase C: out_partial = C @ Wo_slice via C^T blocks as lhsT.
"""

import numpy as np

import concourse.bass as bass
import concourse.mybir as mybir
import concourse.tile as tile
from concourse import bacc
from concourse.bass_utils import run_bass_kernel_spmd

B = 4
S = 2048
D = 2048
H = 16
DH = 128
HPC = 8          # heads per core
DHG = HPC * DH   # 1024: head-group width per core
KT = D // 128    # 16 k-tiles over the model dim
ST = S // 128    # 16 s-tiles
QC = S // 512    # 4 q-chunks
SCALE = 1.0 / np.sqrt(DH)
NEG = -1.0e30

F32 = mybir.dt.float32
F16 = mybir.dt.float16


def _build_nc():
    nc = bacc.Bacc(None, target_bir_lowering=False)

    xT = nc.declare_dram_parameter("xT", [D, S], F16, isOutput=False)
    # wq/wk host-pregathered to [HPC*128, KT*128]: row t*128+p, col n*128+m
    # = Wq[n*128+p, t*128+m] so each head-tile's weights DMA contiguously
    wq = nc.declare_dram_parameter("wq", [DHG, D], F16, isOutput=False)
    wk = nc.declare_dram_parameter("wk", [DHG, D], F16, isOutput=False)
    wv = nc.declare_dram_parameter("wv", [D, DHG], F16, isOutput=False)
    wo = nc.declare_dram_parameter("wo", [DHG, D], F16, isOutput=False)
    bqT = nc.declare_dram_parameter("bqT", [128, HPC], F32, isOutput=False)
    bkT = nc.declare_dram_parameter("bkT", [128, HPC], F32, isOutput=False)
    bvb = nc.declare_dram_parameter("bvb", [128, DHG], F32, isOutput=False)
    cmask = nc.declare_dram_parameter("cmask", [128, 896], F32, isOutput=False)
    out = nc.declare_dram_parameter("out", [S, D], F32, isOutput=True)

    with tile.TileContext(nc) as tc:
        _emit(nc, tc, xT, wq, wk, wv, wo, bqT, bkT, bvb, cmask, out)
    nc.compile()
    return nc


def _emit(nc, tc, xT, wq, wk, wv, wo, bqT, bkT, bvb, cmask, out):
    with (
        tc.tile_pool(name="const", bufs=1) as const,
        tc.tile_pool(name="dram", bufs=1, space="DRAM") as dram,
        tc.tile_pool(name="qkpre", bufs=2) as qkpre,
    ):
        qt_d = dram.tile([DHG, S], F16)
        kt_d = dram.tile([DHG, S], F16)
        v_d = dram.tile([S, DHG], F16)
        ct_d = dram.tile([DHG, S], F16)

        cm_sb = const.tile([128, 896], F32)
        nc.sync.dma_start(out=cm_sb, in_=cmask[:, :])
        bq_sb = const.tile([128, HPC], F32)
        nc.sync.dma_start(out=bq_sb, in_=bqT[:, :])
        bk_sb = const.tile([128, HPC], F32)
        nc.sync.dma_start(out=bk_sb, in_=bkT[:, :])
        bv_sb = const.tile([128, DHG], F32)
        nc.sync.dma_start(out=bv_sb, in_=bvb[:, :])
        ones_f32 = const.tile([128, 128], F32)
        nc.vector.memset(ones_f32, 1.0)
        ones128 = const.tile([128, 128], F16)
        nc.vector.tensor_copy(out=ones128, in_=ones_f32)

        v_r = v_d[:, :].rearrange("(n p) d -> p n d", p=128)
        ct_r = ct_d[:, :].rearrange("(n p) m -> p n m", p=128)
        head0 = {}   # prefetched head-0 tiles, loaded during phase A
        ct_pre = {}  # prefetched phase-C ct tiles, loaded during phase B

        # ---------------- Phase A: projections, spilled to DRAM -------------
        wv_r = wv.rearrange("(n p) m -> p n m", p=128)

        with (
            tc.tile_pool(name="xts", bufs=3) as xtp,
            tc.tile_pool(name="wqk", bufs=2) as wqk,
            tc.tile_pool(name="wvp", bufs=2) as wvp,
            tc.tile_pool(name="apsum", bufs=8, space="PSUM") as aps,
            tc.tile_pool(name="astage", bufs=4) as ast,
        ):
            for sh in range(2):
                s0 = sh * (S // 2)
                # weight tiles prefetched (depth 2) ahead of the bulk xT DMAs
                seq = [(w, b, dst, t)
                       for w, b, dst in ((wq, bq_sb, qt_d), (wk, bk_sb, kt_d))
                       for t in range(HPC)]
                w_tiles = {}

                def w_prefetch(i):
                    if i < len(seq):
                        w, _, _, t = seq[i]
                        w_sb = wqk.tile([128, KT, 128], F16, tag="wqk",
                                        name=f"w_sb{i % 2}")
                        nc.sync.dma_start(
                            out=w_sb,
                            in_=w[t * 128 : (t + 1) * 128, :]
                            .rearrange("p (n m) -> p n m", m=128),
                        )
                        w_tiles[i] = w_sb

                w_prefetch(0)

                # xT half as two sub-tiles of 8 k-tiles each (bufs=3 lets the
                # next half's first sub-tile prefetch during this half).
                xt_lo = xtp.tile([128, 8, S // 2], F16, tag="xts")
                xt_hi = xtp.tile([128, 8, S // 2], F16, tag="xts")

                def xt_blk(kd):
                    t = xt_lo if kd < 8 else xt_hi
                    return t[:, kd % 8, :]

                for kd in range(KT):
                    nc.sync.dma_start(
                        out=xt_blk(kd),
                        in_=xT[kd * 128 : (kd + 1) * 128, s0 : s0 + S // 2],
                    )

                # Q^T and K^T: psum[dh 128, s 512] = sum_kd Wblk^T @ xTblk
                for i, (w, b_sb, dst, t) in enumerate(seq):
                    w_sb = w_tiles.pop(i)
                    w_prefetch(i + 1)
                    for sc in range(2):
                        psum = aps.tile([128, 512], F32, tag="apsum", name="qk_ps")
                        for kd in range(KT):
                            nc.tensor.matmul(
                                psum,
                                w_sb[:, kd, :],
                                xt_blk(kd)[:, sc * 512 : (sc + 1) * 512],
                                start=(kd == 0),
                                stop=(kd == KT - 1),
                            )
                        stg = ast.tile([128, 512], F16, tag="astage")
                        nc.vector.tensor_scalar_add(
                            out=stg, in0=psum, scalar1=b_sb[:, t : t + 1]
                        )
                        nc.gpsimd.dma_start(
                            out=dst[
                                t * 128 : (t + 1) * 128,
                                s0 + sc * 512 : s0 + (sc + 1) * 512,
                            ],
                            in_=stg,
                        )

                if sh == 1:
                    qt0 = qkpre.tile([128, S], F16, tag="qt", name="qt0")
                    nc.sync.dma_start(out=qt0, in_=qt_d[0:128, :])
                    kt0 = qkpre.tile([128, S], F16, tag="kt", name="kt0")
                    nc.sync.dma_start(out=kt0, in_=kt_d[0:128, :])
                    head0["qt"] = qt0
                    head0["kt"] = kt0

                # V: psum[s 128, dh 512] = sum_kd xTblk^T @ Wvblk.
                # wv streamed two k-tiles per DMA; 4 s-tiles accumulate at once.
                for t2 in range(2):
                    # wv half-chunks (8 k-tiles each), double-buffered; all 8
                    # s-tiles of this half accumulate in one kd sweep
                    wv_lo = wvp.tile([128, 8, 512], F16, tag="wvp", name="wv_lo")
                    nc.sync.dma_start(
                        out=wv_lo, in_=wv_r[:, 0:8, t2 * 512 : (t2 + 1) * 512]
                    )
                    wv_hi = wvp.tile([128, 8, 512], F16, tag="wvp", name="wv_hi")
                    nc.sync.dma_start(
                        out=wv_hi, in_=wv_r[:, 8:16, t2 * 512 : (t2 + 1) * 512]
                    )
                    psums = [
                        aps.tile([128, 512], F32, tag="apsum", name=f"vps{si}")
                        for si in range(8)
                    ]
                    for kd in range(KT):
                        wv_blk = wv_lo if kd < 8 else wv_hi
                        for si in range(8):
                            nc.tensor.matmul(
                                psums[si],
                                xt_blk(kd)[:, si * 128 : (si + 1) * 128],
                                wv_blk[:, kd % 8, :],
                                start=(kd == 0),
                                stop=(kd == KT - 1),
                            )
                    for si in range(8):
                        stg = ast.tile([128, 512], F16, tag="astage")
                        nc.vector.tensor_tensor(
                            out=stg,
                            in0=psums[si],
                            in1=bv_sb[:, t2 * 512 : (t2 + 1) * 512],
                            op=mybir.AluOpType.add,
                        )
                        nc.gpsimd.dma_start(
                            out=v_d[
                                s0 + si * 128 : s0 + (si + 1) * 128,
                                t2 * 512 : (t2 + 1) * 512,
                            ],
                            in_=stg,
                        )

        # ---------------- Phase B: per-head attention ------------------------
        wo_r = wo.rearrange("(n p) m -> p n m", p=128)
        wop_cm = tc.tile_pool(name="wop", bufs=1)
        wop = wop_cm.__enter__()
        ctin_cm = tc.tile_pool(name="ctin", bufs=3)
        ctin = ctin_cm.__enter__()
        with (
            tc.tile_pool(name="vpool", bufs=2) as vpool,
            tc.tile_pool(name="ct", bufs=4) as ctpool,
            tc.tile_pool(name="ptile", bufs=6) as ppool,
            tc.tile_pool(name="msk", bufs=3) as mpool,
            tc.tile_pool(name="rcp", bufs=2) as rcpool,
            tc.tile_pool(name="pscore", bufs=3, space="PSUM") as pscore,
            tc.tile_pool(name="pctx", bufs=2, space="PSUM") as pctx,
            tc.tile_pool(name="psum2", bufs=2, space="PSUM") as psums,
        ):
            wo_sb = wop.tile([128, HPC, D], F16)
            for h in range(HPC):
                if h == 0:
                    qt_sb = head0["qt"]
                    kt_sb = head0["kt"]
                else:
                    qt_sb = qkpre.tile([128, S], F16, tag="qt", name="qt_sb")
                    nc.sync.dma_start(out=qt_sb, in_=qt_d[h * 128 : (h + 1) * 128, :])
                    kt_sb = qkpre.tile([128, S], F16, tag="kt", name="kt_sb")
                    nc.sync.dma_start(out=kt_sb, in_=kt_d[h * 128 : (h + 1) * 128, :])
                v_sb = vpool.tile([128, ST, 128], F16, tag="v", name="v_sb")
                nc.sync.dma_start(out=v_sb, in_=v_r[:, :, h * 128 : (h + 1) * 128])
                # spread the 4MB Wo load through phase B on the idle sync queue
                nc.sync.dma_start(out=wo_sb[:, h, :], in_=wo_r[:, h, :])

                for qc in range(QC):
                    nkt = 4 * qc + 4
                    # diagonal tiles first: their longer PE->DVE->ACT chains
                    # start early and overlap with the full tiles' stream
                    order = list(range(4 * qc, nkt)) + list(range(4 * qc))
                    psum_c = pctx.tile([128, 512], F32)
                    psum_s = psums.tile([128, 512], F32)

                    def scores(kt_i):
                        # diagonal tile j has valid columns only at qq >= 128j:
                        # compute just that [128, 512-128j] strip
                        j = kt_i - 4 * qc
                        off = 128 * j if j > 0 else 0
                        ps_t = pscore.tile([128, 512], F32, tag="ps_t")
                        nc.tensor.matmul(
                            ps_t[:, off:],
                            kt_sb[:, kt_i * 128 : (kt_i + 1) * 128],
                            qt_sb[:, qc * 512 + off : (qc + 1) * 512],
                            start=True,
                            stop=True,
                        )
                        p_t = ppool.tile([128, 512], F16, tag="p_t")
                        if j >= 0:
                            msk = mpool.tile([128, 512], F32, tag="msk")
                            nc.vector.tensor_tensor(
                                out=msk[:, off:],
                                in0=ps_t[:, off:],
                                in1=cm_sb[:, 384 : 896 - off],
                                op=mybir.AluOpType.add,
                            )
                            src = msk
                        else:
                            src = ps_t
                        nc.scalar.activation(
                            out=p_t[:, off:],
                            in_=src[:, off:],
                            func=mybir.ActivationFunctionType.Exp,
                            scale=float(SCALE),
                        )
                        return p_t, off

                    def ctx(idx, kt_i, p_t, off):
                        nc.tensor.matmul(
                            psum_c[:, off:],
                            v_sb[:, kt_i, :],
                            p_t[:, off:],
                            start=(idx == 0),
                            stop=(idx == nkt - 1),
                        )
                        # every psum_s row accumulates the per-q denominator
                        nc.tensor.matmul(
                            psum_s[:, off:],
                            ones128,
                            p_t[:, off:],
                            start=(idx == 0),
                            stop=(idx == nkt - 1),
                        )

                    # software-pipeline scores/exp ahead of ctx by one tile
                    prev = None
                    for idx, kt_i in enumerate(order):
                        p_t, off = scores(kt_i)
                        if prev is not None:
                            ctx(idx - 1, prev[0], prev[1], prev[2])
                        prev = (kt_i, p_t, off)
                    ctx(nkt - 1, prev[0], prev[1], prev[2])

                    recip = rcpool.tile([128, 512], F32, tag="rcp")
                    nc.vector.reciprocal_approx_fast(out=recip, in_=psum_s)
                    ct = ctpool.tile([128, 512], F16, tag="ct")
                    nc.vector.tensor_tensor(
                        out=ct,
                        in0=psum_c,
                        in1=recip,
                        op=mybir.AluOpType.mult,
                    )
                    nc.gpsimd.dma_start(
                        out=ct_d[h * 128 : (h + 1) * 128, qc * 512 : (qc + 1) * 512],
                        in_=ct,
                    )
                    if h == HPC - 1 and qc < 2:
                        pre = ctin.tile([128, HPC, 128], F16, tag="ctin",
                                        name=f"ctpre{qc}")
                        nc.sync.dma_start(
                            out=pre,
                            in_=ct_r[:, :, qc * 4 * 128 : (qc * 4 + 1) * 128],
                        )
                        ct_pre[qc * 4] = pre

        # ---------------- Phase C: output projection -------------------------
        with (
            tc.tile_pool(name="opsum", bufs=4, space="PSUM") as ops,
            tc.tile_pool(name="ostage", bufs=4) as ost,
        ):
            for st in range(ST):
                if st in ct_pre:
                    ct_sb = ct_pre[st]
                else:
                    ct_sb = ctin.tile([128, HPC, 128], F16, tag="ctin",
                                      name="ct_sb")
                    nc.sync.dma_start(
                        out=ct_sb, in_=ct_r[:, :, st * 128 : (st + 1) * 128]
                    )
                for ncol in range(4):
                    psum = ops.tile([128, 512], F32)
                    for hh in range(HPC):
                        nc.tensor.matmul(
                            psum,
                            ct_sb[:, hh, :],
                            wo_sb[:, hh, ncol * 512 : (ncol + 1) * 512],
                            start=(hh == 0),
                            stop=(hh == HPC - 1),
                        )
                    o_sb = ost.tile([128, 512], F32, tag="ostage")
                    nc.scalar.activation(
                        out=o_sb, in_=psum, func=mybir.ActivationFunctionType.Copy
                    )
                    nc.gpsimd.dma_start(
                        out=out[
                            st * 128 : (st + 1) * 128,
                            ncol * 512 : (ncol + 1) * 512,
                        ],
                        in_=o_sb,
                    )
        ctin_cm.__exit__(None, None, None)
        wop_cm.__exit__(None, None, None)


ctpool_tiles = {}

_NC = None


def _get_nc():
    global _NC
    if _NC is None:
        ctpool_tiles.clear()
        _NC = _build_nc()
    return _NC


def _host_prep(input_sequences, Wq, bq, Wk, bk, Wv, bv, Wo, bo):
    """Build per-core input maps."""
    x = np.asarray(input_sequences, dtype=np.float32)
    cm = np.full((128, 896), NEG, dtype=np.float32)
    kk = np.arange(128)[:, None]
    uu = np.arange(896)[None, :]
    cm[kk <= uu - 384] = 0.0

    in_maps = []
    for c in range(8):
        b, g = divmod(c, 2)
        sl = slice(g * DHG, (g + 1) * DHG)
        wq_c = np.ascontiguousarray(
            np.asarray(Wq[:, sl], dtype=np.float32)
            .reshape(KT, 128, HPC, 128).transpose(2, 1, 0, 3).reshape(DHG, D)
        ).astype(np.float16)
        wk_c = np.ascontiguousarray(
            np.asarray(Wk[:, sl], dtype=np.float32)
            .reshape(KT, 128, HPC, 128).transpose(2, 1, 0, 3).reshape(DHG, D)
        ).astype(np.float16)
        wv_c = np.ascontiguousarray(Wv[:, sl]).astype(np.float16)
        wo_c = np.ascontiguousarray(Wo[sl, :]).astype(np.float16)
        in_maps.append({
            "xT": np.ascontiguousarray(x[b].T).astype(np.float16),
            "wq": wq_c,
            "wk": wk_c,
            "wv": wv_c,
            "wo": wo_c,
            "bqT": np.ascontiguousarray(
                np.asarray(bq[sl], dtype=np.float32).reshape(HPC, 128).T
            ),
            "bkT": np.ascontiguousarray(
                np.asarray(bk[sl], dtype=np.float32).reshape(HPC, 128).T
            ),
            "bvb": np.ascontiguousarray(
                np.broadcast_to(np.asarray(bv[sl], dtype=np.float32), (128, DHG))
            ),
            "cmask": cm,
        })
    return in_maps


def kernel(input_sequences, Wq, bq, Wk, bk, Wv, bv, Wo, bo, _trace=False):
    nc = _get_nc()
    in_maps = _host_prep(input_sequences, Wq, bq, Wk, bk, Wv, bv, Wo, bo)
    res = run_bass_kernel_spmd(nc, in_maps, list(range(8)), trace=_trace)
    bo32 = np.asarray(bo, dtype=np.float32)
    out = np.empty((B, S, D), dtype=np.float32)
    for b in range(B):
        out[b] = res.results[2 * b]["out"] + res.results[2 * b + 1]["out"] + bo32
    if _trace:
        kernel.last_exec_time_ns = res.exec_time_ns
    return out


# revision 8
# speedup vs baseline: 1.2972x; 1.0122x over previous
"""Causal multi-head attention on 8 trn2 NeuronCores.

Problem: B=4, S=2048, D=2048, H=16 heads, head_dim=128, causal softmax,
torch-style Linear projections (W stored [in, out]).

Sharding: core c handles batch b = c//2 and head-group g = c%2
(8 heads = 1024 output columns of Wq/Wk/Wv, 1024 rows of Wo).
Each core produces a partial output [S, D]; host sums the two
head-group partials per batch and adds bo.

All matmul operands fp16 (PSUM accumulation stays fp32): fp16 streams
at 1 row/cycle on the PE vs fp32r's ~1.25, and halves DMA/SBUF bytes.

Per-core device pipeline:
  Phase A: from xT (host-pretransposed [D, S]) compute
           Q^T, K^T [1024, S] (spilled to DRAM scratch) and V [S, 1024]
           (kept SBUF-resident: 32KB/partition in fp16).
  Phase B: per head h, per 512-wide q-chunk: k-tile PAIRS:
           scores^T strips [128 k, 512 q] = K_h Q_h^T into a 2-bank
           psum pair, one exp per pair (psum -> fp16 SBUF, no mask),
           causal upper-triangle + garbage zeroed by one gpsimd
           affine_select per diagonal pair (exact: zeros contribute
           nothing downstream), ctx^T accumulation C^T = V_h^T @ P^T,
           denominator accumulated on DVE in fp16 (one ones-matmul per
           q-chunk instead of one per tile), normalize into
           SBUF-resident C^T (32KB/partition).
  Phase C: out_partial = C @ Wo_slice straight from SBUF ct tiles.
"""

import numpy as np

import concourse.bass as bass
import concourse.mybir as mybir
import concourse.tile as tile
from concourse import bacc
from concourse.bass_utils import run_bass_kernel_spmd

B = 4
S = 2048
D = 2048
H = 16
DH = 128
HPC = 8          # heads per core
DHG = HPC * DH   # 1024: head-group width per core
KT = D // 128    # 16 k-tiles over the model dim
ST = S // 128    # 16 s-tiles
QC = S // 512    # 4 q-chunks
SCALE = 1.0 / np.sqrt(DH)

F32 = mybir.dt.float32
F16 = mybir.dt.float16


def _build_nc():
    nc = bacc.Bacc(None, target_bir_lowering=False)

    xT = nc.declare_dram_parameter("xT", [D, S], F16, isOutput=False)
    # wq/wk host-pregathered to [HPC*128, KT*128]: row t*128+p, col n*128+m
    # = Wq[n*128+p, t*128+m] so each head-tile's weights DMA contiguously
    wq = nc.declare_dram_parameter("wq", [DHG, D], F16, isOutput=False)
    wk = nc.declare_dram_parameter("wk", [DHG, D], F16, isOutput=False)
    wv = nc.declare_dram_parameter("wv", [D, DHG], F16, isOutput=False)
    wo = nc.declare_dram_parameter("wo", [DHG, D], F16, isOutput=False)
    bqT = nc.declare_dram_parameter("bqT", [128, HPC], F32, isOutput=False)
    bkT = nc.declare_dram_parameter("bkT", [128, HPC], F32, isOutput=False)
    bvb = nc.declare_dram_parameter("bvb", [128, DHG], F32, isOutput=False)
    out = nc.declare_dram_parameter("out", [S, D], F32, isOutput=True)

    with tile.TileContext(nc) as tc:
        _emit(nc, tc, xT, wq, wk, wv, wo, bqT, bkT, bvb, out)
    nc.compile()
    return nc


def _emit(nc, tc, xT, wq, wk, wv, wo, bqT, bkT, bvb, out):
    with (
        tc.tile_pool(name="const", bufs=1) as const,
        tc.tile_pool(name="dram", bufs=1, space="DRAM") as dram,
        tc.tile_pool(name="qkpre", bufs=2) as qkpre,
        tc.tile_pool(name="vfull", bufs=1) as vfull,
    ):
        qt_d = dram.tile([DHG, S], F16)
        kt_d = dram.tile([DHG, S], F16)

        bq_sb = const.tile([128, HPC], F32)
        nc.sync.dma_start(out=bq_sb, in_=bqT[:, :])
        bk_sb = const.tile([128, HPC], F32)
        nc.sync.dma_start(out=bk_sb, in_=bkT[:, :])
        bv_sb = const.tile([128, DHG], F32)
        nc.sync.dma_start(out=bv_sb, in_=bvb[:, :])
        ones_f32 = const.tile([128, 128], F32)
        nc.vector.memset(ones_f32, 1.0)
        ones128 = const.tile([128, 128], F16)
        nc.vector.tensor_copy(out=ones128, in_=ones_f32)

        # V [S, DHG] stays in SBUF for the whole kernel (ct_full and wo_sb
        # allocate lazily at phase B so their space is free during phase A)
        v_full = vfull.tile([128, ST, DHG], F16)

        head0 = {}  # head-0 q/k tiles, prefetched during phase A

        # ---------------- Phase A: projections -------------------------------
        wv_r = wv.rearrange("(n p) m -> p n m", p=128)

        with (
            tc.tile_pool(name="xts", bufs=3) as xtp,
            tc.tile_pool(name="wqk", bufs=2) as wqk,
            tc.tile_pool(name="wvp", bufs=4) as wvp,
            tc.tile_pool(name="apsum", bufs=8, space="PSUM") as aps,
            tc.tile_pool(name="astage", bufs=4) as ast,
        ):
            # wv is half-batch-independent: load all four chunks once, on the
            # otherwise-idle scalar queue so they never contend with xT/wq DMAs
            wv_chunks = {}
            for kdh in range(2):
                for t2 in range(2):
                    wvt = wvp.tile([128, 8, 512], F16, tag="wvp",
                                   name=f"wv{kdh}{t2}")
                    nc.scalar.dma_start(
                        out=wvt,
                        in_=wv_r[:, kdh * 8 : (kdh + 1) * 8,
                                 t2 * 512 : (t2 + 1) * 512],
                    )
                    wv_chunks[(kdh, t2)] = wvt

            for sh in range(2):
                s0 = sh * (S // 2)
                # weight tiles prefetched (depth 2) ahead of the bulk xT DMAs
                seq = [(w, b, dst, t)
                       for w, b, dst in ((wq, bq_sb, qt_d), (wk, bk_sb, kt_d))
                       for t in range(HPC)]
                w_tiles = {}

                def w_prefetch(i):
                    if i < len(seq):
                        w, _, _, t = seq[i]
                        w_sb = wqk.tile([128, KT, 128], F16, tag="wqk",
                                        name=f"w_sb{i % 2}")
                        nc.sync.dma_start(
                            out=w_sb,
                            in_=w[t * 128 : (t + 1) * 128, :]
                            .rearrange("p (n m) -> p n m", m=128),
                        )
                        w_tiles[i] = w_sb

                w_prefetch(0)

                # xT half as two sub-tiles of 8 k-tiles each, DMA'd per k-tile
                # so the first matmul starts as soon as tile 0 lands
                xt_lo = xtp.tile([128, 8, S // 2], F16, tag="xts")
                xt_hi = xtp.tile([128, 8, S // 2], F16, tag="xts")

                def xt_blk(kd):
                    t = xt_lo if kd < 8 else xt_hi
                    return t[:, kd % 8, :]

                for kd in range(KT):
                    nc.sync.dma_start(
                        out=xt_blk(kd),
                        in_=xT[kd * 128 : (kd + 1) * 128, s0 : s0 + S // 2],
                    )

                # Q^T and K^T: psum[dh 128, s 512] = sum_kd Wblk^T @ xTblk
                for i, (w, b_sb, dst, t) in enumerate(seq):
                    w_sb = w_tiles.pop(i)
                    w_prefetch(i + 1)
                    for sc in range(2):
                        psum = aps.tile([128, 512], F32, tag="apsum", name="qk_ps")
                        for kd in range(KT):
                            nc.tensor.matmul(
                                psum,
                                w_sb[:, kd, :],
                                xt_blk(kd)[:, sc * 512 : (sc + 1) * 512],
                                start=(kd == 0),
                                stop=(kd == KT - 1),
                            )
                        stg = ast.tile([128, 512], F16, tag="astage")
                        nc.vector.tensor_scalar_add(
                            out=stg, in0=psum, scalar1=b_sb[:, t : t + 1]
                        )
                        nc.gpsimd.dma_start(
                            out=dst[
                                t * 128 : (t + 1) * 128,
                                s0 + sc * 512 : s0 + (sc + 1) * 512,
                            ],
                            in_=stg,
                        )

                if sh == 1:
                    qt0 = qkpre.tile([128, S], F16, tag="qt", name="qt0")
                    nc.sync.dma_start(out=qt0, in_=qt_d[0:128, :])
                    kt0 = qkpre.tile([128, S], F16, tag="kt", name="kt0")
                    nc.sync.dma_start(out=kt0, in_=kt_d[0:128, :])
                    head0["qt"] = qt0
                    head0["kt"] = kt0

                # V: psum[s 128, dh 512] = sum_kd xTblk^T @ Wvblk; bias-added
                # result written straight into the SBUF-resident v_full
                for t2 in range(2):
                    psums = [
                        aps.tile([128, 512], F32, tag="apsum", name=f"vps{si}")
                        for si in range(8)
                    ]
                    for kd in range(KT):
                        wv_blk = wv_chunks[(kd // 8, t2)]
                        for si in range(8):
                            nc.tensor.matmul(
                                psums[si],
                                xt_blk(kd)[:, si * 128 : (si + 1) * 128],
                                wv_blk[:, kd % 8, :],
                                start=(kd == 0),
                                stop=(kd == KT - 1),
                            )
                    for si in range(8):
                        nc.vector.tensor_tensor(
                            out=v_full[:, sh * 8 + si,
                                       t2 * 512 : (t2 + 1) * 512],
                            in0=psums[si],
                            in1=bv_sb[:, t2 * 512 : (t2 + 1) * 512],
                            op=mybir.AluOpType.add,
                        )

        # ---------------- Phase B: per-head attention ------------------------
        wo_r = wo.rearrange("(n p) m -> p n m", p=128)
        # ct_full / wo_sb pools open only now (span B and C) so their 64KB
        # per partition is free during phase A
        ctfull_cm = tc.tile_pool(name="ctfull", bufs=1)
        ctfull = ctfull_cm.__enter__()
        wop_cm = tc.tile_pool(name="wop", bufs=1)
        wop = wop_cm.__enter__()
        with (
            tc.tile_pool(name="ptile", bufs=4) as ppool,
            tc.tile_pool(name="pacc", bufs=2) as paccp,
            tc.tile_pool(name="rcp", bufs=2) as rcpool,
            tc.tile_pool(name="pscore", bufs=2, space="PSUM") as pscore,
            tc.tile_pool(name="pctx", bufs=2, space="PSUM") as pctx,
            tc.tile_pool(name="psum2", bufs=2, space="PSUM") as psums,
        ):
            ct_full = ctfull.tile([128, HPC, S], F16)
            wo_sb = wop.tile([128, HPC, D], F16)
            for h in range(HPC):
                if h == 0:
                    qt_sb = head0["qt"]
                    kt_sb = head0["kt"]
                else:
                    qt_sb = qkpre.tile([128, S], F16, tag="qt", name="qt_sb")
                    nc.sync.dma_start(out=qt_sb, in_=qt_d[h * 128 : (h + 1) * 128, :])
                    kt_sb = qkpre.tile([128, S], F16, tag="kt", name="kt_sb")
                    nc.sync.dma_start(out=kt_sb, in_=kt_d[h * 128 : (h + 1) * 128, :])
                # spread the 4MB Wo load through phase B on the sync queue
                nc.sync.dma_start(out=wo_sb[:, h, :], in_=wo_r[:, h, :])

                for qc in range(QC):
                    nkt = 4 * qc + 4
                    # k-tile pairs; diagonal pairs first so their longer
                    # PE->ACT->GPS chains overlap the full pairs' stream
                    pairs = [(4 * qc, 4 * qc + 1), (4 * qc + 2, 4 * qc + 3)]
                    pairs += [(t, t + 1) for t in range(0, 4 * qc, 2)]
                    psum_c = pctx.tile([128, 512], F32)
                    p_acc = paccp.tile([128, 512], F16, tag="pacc")

                    def scores(pair):
                        """Strip matmuls into a 2-bank psum pair, one exp,
                        one affine_select for diagonal pairs."""
                        pp = pscore.tile([128, 1024], F32, tag="pp")
                        offs = []
                        for half, kt_i in enumerate(pair):
                            j = kt_i - 4 * qc
                            off = 128 * j if j > 0 else 0
                            nc.tensor.matmul(
                                pp[:, half * 512 + off : (half + 1) * 512],
                                kt_sb[:, kt_i * 128 : (kt_i + 1) * 128],
                                qt_sb[:, qc * 512 + off : (qc + 1) * 512],
                                start=True,
                                stop=True,
                            )
                            offs.append((kt_i, j, off))
                        p_t = ppool.tile([128, 1024], F16, tag="p_t")
                        nc.scalar.activation(
                            out=p_t,
                            in_=pp,
                            func=mybir.ActivationFunctionType.Exp,
                            scale=float(SCALE),
                        )
                        j0 = offs[0][1]
                        if j0 >= 0:
                            # zero the causally-invalid region of both halves
                            # (including exp'd garbage columns): element
                            # (k, half, u) is valid iff u - k - 128*(j0+half)
                            # >= 0
                            nc.gpsimd.affine_select(
                                out=p_t,
                                in_=p_t,
                                pattern=[[-128, 2], [1, 512]],
                                compare_op=mybir.AluOpType.is_ge,
                                fill=0.0,
                                base=-128 * j0,
                                channel_multiplier=-1,
                            )
                        return p_t, offs

                    def ctx(p_t, offs, first, last):
                        for half, (kt_i, j, off) in enumerate(offs):
                            seg = p_t[:, half * 512 + off : (half + 1) * 512]
                            nc.tensor.matmul(
                                psum_c[:, off:],
                                v_full[:, kt_i, h * 128 : (h + 1) * 128],
                                seg,
                                start=(first and half == 0),
                                stop=(last and half == 1),
                            )
                            # fp16 denominator accumulation on DVE (full
                            # width: invalid region is exact zero)
                            full = p_t[:, half * 512 : (half + 1) * 512]
                            if first and half == 0:
                                nc.vector.tensor_copy(out=p_acc, in_=full)
                            else:
                                nc.vector.tensor_tensor(
                                    out=p_acc,
                                    in0=p_acc,
                                    in1=full,
                                    op=mybir.AluOpType.add,
                                )

                    # software-pipeline scores/exp one pair ahead of ctx
                    prev = None
                    for pi, pair in enumerate(pairs):
                        cur = scores(pair)
                        if prev is not None:
                            ctx(prev[0], prev[1], first=(pi == 1), last=False)
                        prev = cur
                    ctx(prev[0], prev[1], first=(len(pairs) == 1), last=True)

                    psum_s = psums.tile([128, 512], F32)
                    nc.tensor.matmul(
                        psum_s, ones128, p_acc, start=True, stop=True
                    )
                    recip = rcpool.tile([128, 512], F32, tag="rcp")
                    nc.vector.reciprocal_approx_fast(out=recip, in_=psum_s)
                    nc.vector.tensor_tensor(
                        out=ct_full[:, h, qc * 512 : (qc + 1) * 512],
                        in0=psum_c,
                        in1=recip,
                        op=mybir.AluOpType.mult,
                    )

        # ---------------- Phase C: output projection -------------------------
        with (
            tc.tile_pool(name="opsum", bufs=4, space="PSUM") as ops,
            tc.tile_pool(name="ostage", bufs=4) as ost,
        ):
            for st in range(ST):
                for ncol in range(4):
                    psum = ops.tile([128, 512], F32)
                    for hh in range(HPC):
                        nc.tensor.matmul(
                            psum,
                            ct_full[:, hh, st * 128 : (st + 1) * 128],
                            wo_sb[:, hh, ncol * 512 : (ncol + 1) * 512],
                            start=(hh == 0),
                            stop=(hh == HPC - 1),
                        )
                    o_sb = ost.tile([128, 512], F32, tag="ostage")
                    nc.scalar.activation(
                        out=o_sb, in_=psum, func=mybir.ActivationFunctionType.Copy
                    )
                    nc.gpsimd.dma_start(
                        out=out[
                            st * 128 : (st + 1) * 128,
                            ncol * 512 : (ncol + 1) * 512,
                        ],
                        in_=o_sb,
                    )
        wop_cm.__exit__(None, None, None)
        ctfull_cm.__exit__(None, None, None)


_NC = None


def _get_nc():
    global _NC
    if _NC is None:
        _NC = _build_nc()
    return _NC


def _host_prep(input_sequences, Wq, bq, Wk, bk, Wv, bv, Wo, bo):
    """Build per-core input maps."""
    x = np.asarray(input_sequences, dtype=np.float32)

    in_maps = []
    for c in range(8):
        b, g = divmod(c, 2)
        sl = slice(g * DHG, (g + 1) * DHG)
        wq_c = np.ascontiguousarray(
            np.asarray(Wq[:, sl], dtype=np.float32)
            .reshape(KT, 128, HPC, 128).transpose(2, 1, 0, 3).reshape(DHG, D)
        ).astype(np.float16)
        wk_c = np.ascontiguousarray(
            np.asarray(Wk[:, sl], dtype=np.float32)
            .reshape(KT, 128, HPC, 128).transpose(2, 1, 0, 3).reshape(DHG, D)
        ).astype(np.float16)
        wv_c = np.ascontiguousarray(Wv[:, sl]).astype(np.float16)
        wo_c = np.ascontiguousarray(Wo[sl, :]).astype(np.float16)
        in_maps.append({
            "xT": np.ascontiguousarray(x[b].T).astype(np.float16),
            "wq": wq_c,
            "wk": wk_c,
            "wv": wv_c,
            "wo": wo_c,
            "bqT": np.ascontiguousarray(
                np.asarray(bq[sl], dtype=np.float32).reshape(HPC, 128).T
            ),
            "bkT": np.ascontiguousarray(
                np.asarray(bk[sl], dtype=np.float32).reshape(HPC, 128).T
            ),
            "bvb": np.ascontiguousarray(
                np.broadcast_to(np.asarray(bv[sl], dtype=np.float32), (128, DHG))
            ),
        })
    return in_maps


def kernel(input_sequences, Wq, bq, Wk, bk, Wv, bv, Wo, bo, _trace=False):
    nc = _get_nc()
    in_maps = _host_prep(input_sequences, Wq, bq, Wk, bk, Wv, bv, Wo, bo)
    res = run_bass_kernel_spmd(nc, in_maps, list(range(8)), trace=_trace)
    bo32 = np.asarray(bo, dtype=np.float32)
    out = np.empty((B, S, D), dtype=np.float32)
    for b in range(B):
        out[b] = res.results[2 * b]["out"] + res.results[2 * b + 1]["out"] + bo32
    if _trace:
        kernel.last_exec_time_ns = res.exec_time_ns
    return out


# revision 10
# speedup vs baseline: 1.3616x; 1.0497x over previous
"""Causal multi-head attention on 8 trn2 NeuronCores.

Problem: B=4, S=2048, D=2048, H=16 heads, head_dim=128, causal softmax,
torch-style Linear projections (W stored [in, out]).

Sharding: core c handles batch b = c//2 and head-group g = c%2
(8 heads = 1024 output columns of Wq/Wk/Wv, 1024 rows of Wo).
Each core produces a partial output [S, D]; host sums the two
head-group partials per batch and adds bo.

All matmul operands fp16 (PSUM accumulation stays fp32): fp16 streams
at 1 row/cycle on the PE vs fp32r's ~1.25, and halves DMA/SBUF bytes.

Per-core device pipeline:
  Phase A: from xT (host-pretransposed [D, S]) compute
           Q^T, K^T [1024, S] (spilled to DRAM scratch) and V [S, 1024]
           (kept SBUF-resident: 32KB/partition in fp16).
  Phase B: per head h, per 512-wide q-chunk: k-tile PAIRS:
           scores^T strips [128 k, 512 q] = K_h Q_h^T into a 2-bank
           psum pair, one exp per pair (psum -> fp16 SBUF, no mask),
           causal upper-triangle + garbage zeroed by one gpsimd
           affine_select per diagonal pair (exact: zeros contribute
           nothing downstream), ctx^T accumulation C^T = V_h^T @ P^T,
           denominator accumulated on DVE in fp16 (one ones-matmul per
           q-chunk instead of one per tile), normalize into
           SBUF-resident C^T (32KB/partition).
  Phase C: out_partial = C @ Wo_slice straight from SBUF ct tiles.
"""

import numpy as np

import concourse.bass as bass
import concourse.mybir as mybir
import concourse.tile as tile
from concourse import bacc
from concourse.bass_utils import run_bass_kernel_spmd

B = 4
S = 2048
D = 2048
H = 16
DH = 128
HPC = 8          # heads per core
DHG = HPC * DH   # 1024: head-group width per core
KT = D // 128    # 16 k-tiles over the model dim
ST = S // 128    # 16 s-tiles
QC = S // 512    # 4 q-chunks
SCALE = 1.0 / np.sqrt(DH)

F32 = mybir.dt.float32
F16 = mybir.dt.float16


def _build_nc():
    nc = bacc.Bacc(None, target_bir_lowering=False)

    xT = nc.declare_dram_parameter("xT", [D, S], F16, isOutput=False)
    # wq/wk host-pregathered to [HPC*128, KT*128]: row t*128+p, col n*128+m
    # = Wq[n*128+p, t*128+m] so each head-tile's weights DMA contiguously
    wq = nc.declare_dram_parameter("wq", [DHG, D], F16, isOutput=False)
    wk = nc.declare_dram_parameter("wk", [DHG, D], F16, isOutput=False)
    wv = nc.declare_dram_parameter("wv", [D, DHG], F16, isOutput=False)
    wo = nc.declare_dram_parameter("wo", [DHG, D], F16, isOutput=False)
    bqT = nc.declare_dram_parameter("bqT", [128, HPC], F32, isOutput=False)
    bkT = nc.declare_dram_parameter("bkT", [128, HPC], F32, isOutput=False)
    bvb = nc.declare_dram_parameter("bvb", [128, DHG], F32, isOutput=False)
    out = nc.declare_dram_parameter("out", [S, D], F32, isOutput=True)

    with tile.TileContext(nc) as tc:
        _emit(nc, tc, xT, wq, wk, wv, wo, bqT, bkT, bvb, out)
    nc.compile()
    return nc


def _emit(nc, tc, xT, wq, wk, wv, wo, bqT, bkT, bvb, out):
    with (
        tc.tile_pool(name="const", bufs=1) as const,
        tc.tile_pool(name="dram", bufs=1, space="DRAM") as dram,
        tc.tile_pool(name="qkpre", bufs=2) as qkpre,
        tc.tile_pool(name="vfull", bufs=1) as vfull,
    ):
        qt_d = dram.tile([DHG, S], F16)
        kt_d = dram.tile([DHG, S], F16)

        bq_sb = const.tile([128, HPC], F32)
        nc.sync.dma_start(out=bq_sb, in_=bqT[:, :])
        bk_sb = const.tile([128, HPC], F32)
        nc.sync.dma_start(out=bk_sb, in_=bkT[:, :])
        bv_sb = const.tile([128, DHG], F32)
        nc.sync.dma_start(out=bv_sb, in_=bvb[:, :])
        ones_f32 = const.tile([128, 128], F32)
        nc.vector.memset(ones_f32, 1.0)
        ones128 = const.tile([128, 128], F16)
        nc.vector.tensor_copy(out=ones128, in_=ones_f32)

        # V [S, DHG] stays in SBUF for the whole kernel (ct_full and wo_sb
        # allocate lazily at phase B so their space is free during phase A)
        v_full = vfull.tile([128, ST, DHG], F16)

        head0 = {}  # head-0 q/k tiles, prefetched during phase A

        # ---------------- Phase A: projections -------------------------------
        wv_r = wv.rearrange("(n p) m -> p n m", p=128)

        with (
            tc.tile_pool(name="xts", bufs=3) as xtp,
            tc.tile_pool(name="wqk", bufs=2) as wqk,
            tc.tile_pool(name="wvp", bufs=4) as wvp,
            tc.tile_pool(name="apsum", bufs=8, space="PSUM") as aps,
            tc.tile_pool(name="astage", bufs=4) as ast,
        ):
            # wv is half-batch-independent: load all four chunks once, on the
            # otherwise-idle scalar queue so they never contend with xT/wq DMAs
            wv_chunks = {}
            for kdh in range(2):
                for t2 in range(2):
                    wvt = wvp.tile([128, 8, 512], F16, tag="wvp",
                                   name=f"wv{kdh}{t2}")
                    nc.scalar.dma_start(
                        out=wvt,
                        in_=wv_r[:, kdh * 8 : (kdh + 1) * 8,
                                 t2 * 512 : (t2 + 1) * 512],
                    )
                    wv_chunks[(kdh, t2)] = wvt

            for sh in range(2):
                s0 = sh * (S // 2)
                # weight tiles prefetched (depth 2) ahead of the bulk xT DMAs
                seq = [(w, b, dst, t)
                       for w, b, dst in ((wq, bq_sb, qt_d), (wk, bk_sb, kt_d))
                       for t in range(HPC)]
                w_tiles = {}

                def w_prefetch(i):
                    if i < len(seq):
                        w, _, _, t = seq[i]
                        w_sb = wqk.tile([128, KT, 128], F16, tag="wqk",
                                        name=f"w_sb{i % 2}")
                        nc.sync.dma_start(
                            out=w_sb,
                            in_=w[t * 128 : (t + 1) * 128, :]
                            .rearrange("p (n m) -> p n m", m=128),
                        )
                        w_tiles[i] = w_sb

                w_prefetch(0)

                # xT half as two sub-tiles of 8 k-tiles each, DMA'd per k-tile
                # so the first matmul starts as soon as tile 0 lands
                xt_lo = xtp.tile([128, 8, S // 2], F16, tag="xts")
                xt_hi = xtp.tile([128, 8, S // 2], F16, tag="xts")

                def xt_blk(kd):
                    t = xt_lo if kd < 8 else xt_hi
                    return t[:, kd % 8, :]

                for kd in range(KT):
                    nc.sync.dma_start(
                        out=xt_blk(kd),
                        in_=xT[kd * 128 : (kd + 1) * 128, s0 : s0 + S // 2],
                    )

                # Q^T and K^T: psum[dh 128, s 512] = sum_kd Wblk^T @ xTblk
                for i, (w, b_sb, dst, t) in enumerate(seq):
                    w_sb = w_tiles.pop(i)
                    w_prefetch(i + 1)
                    for sc in range(2):
                        psum = aps.tile([128, 512], F32, tag="apsum", name="qk_ps")
                        for kd in range(KT):
                            nc.tensor.matmul(
                                psum,
                                w_sb[:, kd, :],
                                xt_blk(kd)[:, sc * 512 : (sc + 1) * 512],
                                start=(kd == 0),
                                stop=(kd == KT - 1),
                            )
                        stg = ast.tile([128, 512], F16, tag="astage")
                        nc.vector.tensor_scalar_add(
                            out=stg, in0=psum, scalar1=b_sb[:, t : t + 1]
                        )
                        nc.gpsimd.dma_start(
                            out=dst[
                                t * 128 : (t + 1) * 128,
                                s0 + sc * 512 : s0 + (sc + 1) * 512,
                            ],
                            in_=stg,
                        )

                if sh == 1:
                    qt0 = qkpre.tile([128, S], F16, tag="qt", name="qt0")
                    nc.sync.dma_start(out=qt0, in_=qt_d[0:128, :])
                    kt0 = qkpre.tile([128, S], F16, tag="kt", name="kt0")
                    nc.sync.dma_start(out=kt0, in_=kt_d[0:128, :])
                    head0["qt"] = qt0
                    head0["kt"] = kt0

                # V: psum[s 128, dh 512] = sum_kd xTblk^T @ Wvblk; bias-added
                # result written straight into the SBUF-resident v_full
                for t2 in range(2):
                    psums = [
                        aps.tile([128, 512], F32, tag="apsum", name=f"vps{si}")
                        for si in range(8)
                    ]
                    for kd in range(KT):
                        wv_blk = wv_chunks[(kd // 8, t2)]
                        for si in range(8):
                            nc.tensor.matmul(
                                psums[si],
                                xt_blk(kd)[:, si * 128 : (si + 1) * 128],
                                wv_blk[:, kd % 8, :],
                                start=(kd == 0),
                                stop=(kd == KT - 1),
                            )
                    for si in range(8):
                        nc.vector.tensor_tensor(
                            out=v_full[:, sh * 8 + si,
                                       t2 * 512 : (t2 + 1) * 512],
                            in0=psums[si],
                            in1=bv_sb[:, t2 * 512 : (t2 + 1) * 512],
                            op=mybir.AluOpType.add,
                        )

        # ---------------- Phase B: per-head attention ------------------------
        wo_r = wo.rearrange("(n p) m -> p n m", p=128)
        # ct_full / wo_sb pools open only now (span B and C) so their 64KB
        # per partition is free during phase A
        ctfull_cm = tc.tile_pool(name="ctfull", bufs=1)
        ctfull = ctfull_cm.__enter__()
        wop_cm = tc.tile_pool(name="wop", bufs=1)
        wop = wop_cm.__enter__()
        with (
            tc.tile_pool(name="ptile", bufs=4) as ppool,
            tc.tile_pool(name="pacc", bufs=2) as paccp,
            tc.tile_pool(name="rcp", bufs=2) as rcpool,
            tc.tile_pool(name="pscore", bufs=2, space="PSUM") as pscore,
            tc.tile_pool(name="pctx", bufs=2, space="PSUM") as pctx,
            tc.tile_pool(name="psum2", bufs=2, space="PSUM") as psums,
        ):
            ct_full = ctfull.tile([128, HPC, S], F16)
            wo_sb = wop.tile([128, HPC, D], F16)

            # Flat (head, qc, pair) pipeline.  Scores run one pair ahead of
            # ctx, crossing qc/head boundaries so the PE never drains at a
            # boundary; per-qc finalization (denominator matmul + recip +
            # normalize) is deferred two pair-units so its inputs are ready.
            units = []
            for h in range(HPC):
                for qc in range(QC):
                    prs = [(4 * qc, 4 * qc + 1), (4 * qc + 2, 4 * qc + 3)]
                    prs += [(t, t + 1) for t in range(0, 4 * qc, 2)]
                    for pi, pair in enumerate(prs):
                        units.append({
                            "h": h, "qc": qc, "pair": pair,
                            "first": pi == 0, "last": pi == len(prs) - 1,
                        })

            qk_tiles = {0: (head0["qt"], head0["kt"])}

            def load_head(h):
                if h in qk_tiles or h >= HPC:
                    return
                qt_sb = qkpre.tile([128, S], F16, tag="qt", name="qt_sb")
                nc.sync.dma_start(out=qt_sb, in_=qt_d[h * 128 : (h + 1) * 128, :])
                kt_sb = qkpre.tile([128, S], F16, tag="kt", name="kt_sb")
                nc.sync.dma_start(out=kt_sb, in_=kt_d[h * 128 : (h + 1) * 128, :])
                qk_tiles[h] = (qt_sb, kt_sb)

            state = {}      # (h, qc) -> dict with psum_c, psum_s, p_acc
            finalizes = []  # (due_iteration, h, qc)

            def scores(u):
                h, qc = u["h"], u["qc"]
                qt_sb, kt_sb = qk_tiles[h]
                pp = pscore.tile([128, 1024], F32, tag="pp")
                offs = []
                for half, kt_i in enumerate(u["pair"]):
                    j = kt_i - 4 * qc
                    off = 128 * j if j > 0 else 0
                    nc.tensor.matmul(
                        pp[:, half * 512 + off : (half + 1) * 512],
                        kt_sb[:, kt_i * 128 : (kt_i + 1) * 128],
                        qt_sb[:, qc * 512 + off : (qc + 1) * 512],
                        start=True,
                        stop=True,
                    )
                    offs.append((kt_i, j, off))
                p_t = ppool.tile([128, 1024], F16, tag="p_t")
                j0 = offs[0][1]
                # the (j2,j3) diagonal pair is valid only from column 256 on
                e0 = 256 if j0 == 2 else 0
                nc.scalar.activation(
                    out=p_t[:, e0:],
                    in_=pp[:, e0:],
                    func=mybir.ActivationFunctionType.Exp,
                    scale=float(SCALE),
                )
                if j0 >= 0:
                    # zero the causally-invalid region of both halves
                    # (including exp'd garbage columns): element (k, half, u)
                    # is valid iff u - k - 128*(j0+half) >= 0
                    nc.gpsimd.affine_select(
                        out=p_t,
                        in_=p_t,
                        pattern=[[-128, 2], [1, 512]],
                        compare_op=mybir.AluOpType.is_ge,
                        fill=0.0,
                        base=-128 * j0,
                        channel_multiplier=-1,
                    )
                u["p_t"] = p_t
                u["offs"] = offs

            def ctx(u):
                h, qc = u["h"], u["qc"]
                if u["first"]:
                    state[(h, qc)] = {
                        "psum_c": pctx.tile([128, 512], F32, name="psum_c"),
                        "psum_s": psums.tile([128, 512], F32, name="psum_s"),
                        "acc_started": False,
                    }
                st_ = state[(h, qc)]
                p_t, offs = u["p_t"], u["offs"]
                diag = offs[0][1] >= 0
                for half, (kt_i, j, off) in enumerate(offs):
                    seg = p_t[:, half * 512 + off : (half + 1) * 512]
                    nc.tensor.matmul(
                        st_["psum_c"][:, off:],
                        v_full[:, kt_i, h * 128 : (h + 1) * 128],
                        seg,
                        start=(u["first"] and half == 0),
                        stop=(u["last"] and half == 1),
                    )
                    if diag:
                        # diagonal tiles: denominator strip matmul on the PE
                        nc.tensor.matmul(
                            st_["psum_s"][:, off:],
                            ones128,
                            seg,
                            start=(u["first"] and half == 0),
                            stop=(qc == 0 and u["last"] and half == 1),
                        )
                    else:
                        # full tiles: fp16 accumulation on the DVE (exact:
                        # one ones-matmul at finalize sums the partitions)
                        full = p_t[:, half * 512 : (half + 1) * 512]
                        if not st_["acc_started"]:
                            st_["p_acc"] = paccp.tile([128, 512], F16,
                                                      tag="pacc", name="p_acc")
                            nc.vector.tensor_copy(out=st_["p_acc"], in_=full)
                            st_["acc_started"] = True
                        else:
                            nc.vector.tensor_tensor(
                                out=st_["p_acc"],
                                in0=st_["p_acc"],
                                in1=full,
                                op=mybir.AluOpType.add,
                            )

            def finalize(h, qc):
                st_ = state.pop((h, qc))
                if st_["acc_started"]:
                    nc.tensor.matmul(
                        st_["psum_s"],
                        ones128,
                        st_["p_acc"],
                        start=False,
                        stop=True,
                    )
                recip = rcpool.tile([128, 512], F32, tag="rcp")
                nc.vector.reciprocal_approx_fast(out=recip, in_=st_["psum_s"])
                nc.vector.tensor_tensor(
                    out=ct_full[:, h, qc * 512 : (qc + 1) * 512],
                    in0=st_["psum_c"],
                    in1=recip,
                    op=mybir.AluOpType.mult,
                )

            for i, u in enumerate(units):
                if u["first"] and u["qc"] == 1:
                    load_head(u["h"] + 1)  # prefetch next head's q/k early
                if u["first"] and u["qc"] == 0:
                    # spread the 4MB Wo load through phase B
                    nc.sync.dma_start(out=wo_sb[:, u["h"], :],
                                      in_=wo_r[:, u["h"], :])
                scores(u)
                while finalizes and finalizes[0][0] <= i:
                    _, fh, fqc = finalizes.pop(0)
                    finalize(fh, fqc)
                if i > 0:
                    pu = units[i - 1]
                    ctx(pu)
                    if pu["last"]:
                        finalizes.append((i + 1, pu["h"], pu["qc"]))
            ctx(units[-1])
            finalizes.append((0, units[-1]["h"], units[-1]["qc"]))
            for _, fh, fqc in finalizes:
                finalize(fh, fqc)

        # ---------------- Phase C: output projection -------------------------
        with (
            tc.tile_pool(name="opsum", bufs=4, space="PSUM") as ops,
            tc.tile_pool(name="ostage", bufs=4) as ost,
        ):
            for st in range(ST):
                for ncol in range(4):
                    psum = ops.tile([128, 512], F32)
                    for hh in range(HPC):
                        nc.tensor.matmul(
                            psum,
                            ct_full[:, hh, st * 128 : (st + 1) * 128],
                            wo_sb[:, hh, ncol * 512 : (ncol + 1) * 512],
                            start=(hh == 0),
                            stop=(hh == HPC - 1),
                        )
                    o_sb = ost.tile([128, 512], F32, tag="ostage")
                    nc.scalar.activation(
                        out=o_sb, in_=psum, func=mybir.ActivationFunctionType.Copy
                    )
                    nc.gpsimd.dma_start(
                        out=out[
                            st * 128 : (st + 1) * 128,
                            ncol * 512 : (ncol + 1) * 512,
                        ],
                        in_=o_sb,
                    )
        wop_cm.__exit__(None, None, None)
        ctfull_cm.__exit__(None, None, None)


_NC = None


def _get_nc():
    global _NC
    if _NC is None:
        _NC = _build_nc()
    return _NC


def _host_prep(input_sequences, Wq, bq, Wk, bk, Wv, bv, Wo, bo):
    """Build per-core input maps."""
    x = np.asarray(input_sequences, dtype=np.float32)

    in_maps = []
    for c in range(8):
        b, g = divmod(c, 2)
        sl = slice(g * DHG, (g + 1) * DHG)
        wq_c = np.ascontiguousarray(
            np.asarray(Wq[:, sl], dtype=np.float32)
            .reshape(KT, 128, HPC, 128).transpose(2, 1, 0, 3).reshape(DHG, D)
        ).astype(np.float16)
        wk_c = np.ascontiguousarray(
            np.asarray(Wk[:, sl], dtype=np.float32)
            .reshape(KT, 128, HPC, 128).transpose(2, 1, 0, 3).reshape(DHG, D)
        ).astype(np.float16)
        wv_c = np.ascontiguousarray(Wv[:, sl]).astype(np.float16)
        wo_c = np.ascontiguousarray(Wo[sl, :]).astype(np.float16)
        in_maps.append({
            "xT": np.ascontiguousarray(x[b].T).astype(np.float16),
            "wq": wq_c,
            "wk": wk_c,
            "wv": wv_c,
            "wo": wo_c,
            "bqT": np.ascontiguousarray(
                np.asarray(bq[sl], dtype=np.float32).reshape(HPC, 128).T
            ),
            "bkT": np.ascontiguousarray(
                np.asarray(bk[sl], dtype=np.float32).reshape(HPC, 128).T
            ),
            "bvb": np.ascontiguousarray(
                np.broadcast_to(np.asarray(bv[sl], dtype=np.float32), (128, DHG))
            ),
        })
    return in_maps


def kernel(input_sequences, Wq, bq, Wk, bk, Wv, bv, Wo, bo, _trace=False):
    nc = _get_nc()
    in_maps = _host_prep(input_sequences, Wq, bq, Wk, bk, Wv, bv, Wo, bo)
    res = run_bass_kernel_spmd(nc, in_maps, list(range(8)), trace=_trace)
    bo32 = np.asarray(bo, dtype=np.float32)
    out = np.empty((B, S, D), dtype=np.float32)
    for b in range(B):
        out[b] = res.results[2 * b]["out"] + res.results[2 * b + 1]["out"] + bo32
    if _trace:
        kernel.last_exec_time_ns = res.exec_time_ns
    return out


# revision 13
# speedup vs baseline: 1.3776x; 1.0117x over previous
"""Causal multi-head attention on 8 trn2 NeuronCores.

Problem: B=4, S=2048, D=2048, H=16 heads, head_dim=128, causal softmax,
torch-style Linear projections (W stored [in, out]).

Sharding: core c handles batch b = c//2 and head-group g = c%2
(8 heads = 1024 output columns of Wq/Wk/Wv, 1024 rows of Wo).
Each core produces a partial output [S, D]; host sums the two
head-group partials per batch and adds bo.

All matmul operands fp16 (PSUM accumulation stays fp32): fp16 streams
at 1 row/cycle on the PE vs fp32r's ~1.25, and halves DMA/SBUF bytes.

Per-core device pipeline:
  Phase A: from xT (host-pretransposed [D, S]) compute
           Q^T, K^T [1024, S] (spilled to DRAM scratch) and V [S, 1024]
           (kept SBUF-resident: 32KB/partition in fp16).
  Phase B: per head h, per 512-wide q-chunk: k-tile PAIRS:
           scores^T strips [128 k, 512 q] = K_h Q_h^T into a 2-bank
           psum pair, one exp per pair (psum -> fp16 SBUF, no mask),
           causal upper-triangle + garbage zeroed by one gpsimd
           affine_select per diagonal pair (exact: zeros contribute
           nothing downstream), ctx^T accumulation C^T = V_h^T @ P^T,
           denominator accumulated on DVE in fp16 (one ones-matmul per
           q-chunk instead of one per tile), normalize into
           SBUF-resident C^T (32KB/partition).
  Phase C: out_partial = C @ Wo_slice straight from SBUF ct tiles.
"""

import numpy as np

import concourse.bass as bass
import concourse.mybir as mybir
import concourse.tile as tile
from concourse import bacc
from concourse.bass_utils import run_bass_kernel_spmd

B = 4
S = 2048
D = 2048
H = 16
DH = 128
HPC = 8          # heads per core
DHG = HPC * DH   # 1024: head-group width per core
KT = D // 128    # 16 k-tiles over the model dim
ST = S // 128    # 16 s-tiles
QC = S // 512    # 4 q-chunks
SCALE = 1.0 / np.sqrt(DH)

F32 = mybir.dt.float32
F16 = mybir.dt.float16


def _build_nc():
    nc = bacc.Bacc(None, target_bir_lowering=False)

    xT = nc.declare_dram_parameter("xT", [D, S], F16, isOutput=False)
    # wq/wk host-pregathered to [HPC*128, KT*128]: row t*128+p, col n*128+m
    # = Wq[n*128+p, t*128+m] so each head-tile's weights DMA contiguously
    wq = nc.declare_dram_parameter("wq", [DHG, D], F16, isOutput=False)
    wk = nc.declare_dram_parameter("wk", [DHG, D], F16, isOutput=False)
    wv = nc.declare_dram_parameter("wv", [D, DHG], F16, isOutput=False)
    wo = nc.declare_dram_parameter("wo", [DHG, D], F16, isOutput=False)
    bqT = nc.declare_dram_parameter("bqT", [128, HPC], F32, isOutput=False)
    bkT = nc.declare_dram_parameter("bkT", [128, HPC], F32, isOutput=False)
    bvb = nc.declare_dram_parameter("bvb", [128, DHG], F32, isOutput=False)
    out = nc.declare_dram_parameter("out", [S, D], F32, isOutput=True)

    with tile.TileContext(nc) as tc:
        _emit(nc, tc, xT, wq, wk, wv, wo, bqT, bkT, bvb, out)
    nc.compile()
    return nc


def _emit(nc, tc, xT, wq, wk, wv, wo, bqT, bkT, bvb, out):
    with (
        tc.tile_pool(name="const", bufs=1) as const,
        tc.tile_pool(name="dram", bufs=1, space="DRAM") as dram,
        tc.tile_pool(name="qkpre", bufs=2) as qkpre,
        tc.tile_pool(name="vfull", bufs=1) as vfull,
    ):
        qt_d = dram.tile([DHG, S], F16)
        kt_d = dram.tile([DHG, S], F16)

        bq_sb = const.tile([128, HPC], F32)
        nc.sync.dma_start(out=bq_sb, in_=bqT[:, :])
        bk_sb = const.tile([128, HPC], F32)
        nc.sync.dma_start(out=bk_sb, in_=bkT[:, :])
        bv_sb = const.tile([128, DHG], F32)
        nc.sync.dma_start(out=bv_sb, in_=bvb[:, :])
        ones_f32 = const.tile([128, 128], F32)
        nc.vector.memset(ones_f32, 1.0)
        ones128 = const.tile([128, 128], F16)
        nc.vector.tensor_copy(out=ones128, in_=ones_f32)

        # V [S, DHG] stays in SBUF for the whole kernel (ct_full and wo_sb
        # allocate lazily at phase B so their space is free during phase A)
        v_full = vfull.tile([128, ST, DHG], F16)

        head0 = {}  # head-0 q/k tiles, prefetched during phase A

        # ---------------- Phase A: projections -------------------------------
        wv_r = wv.rearrange("(n p) m -> p n m", p=128)

        with (
            tc.tile_pool(name="xts", bufs=3) as xtp,
            tc.tile_pool(name="wqk", bufs=2) as wqk,
            tc.tile_pool(name="wvp", bufs=4) as wvp,
            tc.tile_pool(name="apsum", bufs=8, space="PSUM") as aps,
            tc.tile_pool(name="astage", bufs=4) as ast,
        ):
            # wv is half-batch-independent: load all four chunks once, on the
            # otherwise-idle scalar queue so they never contend with xT/wq DMAs
            wv_chunks = {}
            for kdh in range(2):
                for t2 in range(2):
                    wvt = wvp.tile([128, 8, 512], F16, tag="wvp",
                                   name=f"wv{kdh}{t2}")
                    nc.scalar.dma_start(
                        out=wvt,
                        in_=wv_r[:, kdh * 8 : (kdh + 1) * 8,
                                 t2 * 512 : (t2 + 1) * 512],
                    )
                    wv_chunks[(kdh, t2)] = wvt

            for sh in range(2):
                s0 = sh * (S // 2)
                # weight tiles prefetched (depth 2) ahead of the bulk xT DMAs
                seq = [(w, b, dst, t)
                       for w, b, dst in ((wq, bq_sb, qt_d), (wk, bk_sb, kt_d))
                       for t in range(HPC)]
                w_tiles = {}

                def w_prefetch(i):
                    if i < len(seq):
                        w, _, _, t = seq[i]
                        w_sb = wqk.tile([128, KT, 128], F16, tag="wqk",
                                        name=f"w_sb{i % 2}")
                        nc.sync.dma_start(
                            out=w_sb,
                            in_=w[t * 128 : (t + 1) * 128, :]
                            .rearrange("p (n m) -> p n m", m=128),
                        )
                        w_tiles[i] = w_sb

                w_prefetch(0)

                # xT half as two sub-tiles of 8 k-tiles each, DMA'd per k-tile
                # so the first matmul starts as soon as tile 0 lands
                xt_lo = xtp.tile([128, 8, S // 2], F16, tag="xts")
                xt_hi = xtp.tile([128, 8, S // 2], F16, tag="xts")

                def xt_blk(kd):
                    t = xt_lo if kd < 8 else xt_hi
                    return t[:, kd % 8, :]

                # xT loads go on the gpsimd queue so the w_sb prefetch chain
                # on the sync queue is never stuck behind them
                for kd in range(KT):
                    nc.gpsimd.dma_start(
                        out=xt_blk(kd),
                        in_=xT[kd * 128 : (kd + 1) * 128, s0 : s0 + S // 2],
                    )

                # Q^T and K^T: psum[dh 128, s 512] = sum_kd Wblk^T @ xTblk
                for i, (w, b_sb, dst, t) in enumerate(seq):
                    w_sb = w_tiles.pop(i)
                    w_prefetch(i + 1)
                    for sc in range(2):
                        psum = aps.tile([128, 512], F32, tag="apsum", name="qk_ps")
                        for kd in range(KT):
                            nc.tensor.matmul(
                                psum,
                                w_sb[:, kd, :],
                                xt_blk(kd)[:, sc * 512 : (sc + 1) * 512],
                                start=(kd == 0),
                                stop=(kd == KT - 1),
                            )
                        stg = ast.tile([128, 512], F16, tag="astage")
                        nc.vector.tensor_scalar_add(
                            out=stg, in0=psum, scalar1=b_sb[:, t : t + 1]
                        )
                        nc.gpsimd.dma_start(
                            out=dst[
                                t * 128 : (t + 1) * 128,
                                s0 + sc * 512 : s0 + (sc + 1) * 512,
                            ],
                            in_=stg,
                        )

                if sh == 1:
                    qt0 = qkpre.tile([128, S], F16, tag="qt", name="qt0")
                    nc.sync.dma_start(out=qt0, in_=qt_d[0:128, :])
                    kt0 = qkpre.tile([128, S], F16, tag="kt", name="kt0")
                    nc.sync.dma_start(out=kt0, in_=kt_d[0:128, :])
                    head0["qt"] = qt0
                    head0["kt"] = kt0

                # V: psum[s 128, dh 512] = sum_kd xTblk^T @ Wvblk; bias-added
                # result written straight into the SBUF-resident v_full.
                # si-groups of 4 so each group's DVE drain overlaps the next
                # group's matmul sweep (and the tail drain is short).
                for t2 in range(2):
                    for sg in range(2):
                        psums = [
                            aps.tile([128, 512], F32, tag="apsum",
                                     name=f"vps{si}")
                            for si in range(4)
                        ]
                        for kd in range(KT):
                            wv_blk = wv_chunks[(kd // 8, t2)]
                            for si in range(4):
                                nc.tensor.matmul(
                                    psums[si],
                                    xt_blk(kd)[:, (sg * 4 + si) * 128 :
                                               (sg * 4 + si + 1) * 128],
                                    wv_blk[:, kd % 8, :],
                                    start=(kd == 0),
                                    stop=(kd == KT - 1),
                                )
                        for si in range(4):
                            nc.vector.tensor_tensor(
                                out=v_full[:, sh * 8 + sg * 4 + si,
                                           t2 * 512 : (t2 + 1) * 512],
                                in0=psums[si],
                                in1=bv_sb[:, t2 * 512 : (t2 + 1) * 512],
                                op=mybir.AluOpType.add,
                            )

        # ---------------- Phase B: per-head attention ------------------------
        wo_r = wo.rearrange("(n p) m -> p n m", p=128)
        # ct_full / wo_sb pools open only now (span B and C) so their 64KB
        # per partition is free during phase A
        ctfull_cm = tc.tile_pool(name="ctfull", bufs=1)
        ctfull = ctfull_cm.__enter__()
        wop_cm = tc.tile_pool(name="wop", bufs=1)
        wop = wop_cm.__enter__()
        with (
            tc.tile_pool(name="ptile", bufs=4) as ppool,
            tc.tile_pool(name="pacc", bufs=2) as paccp,
            tc.tile_pool(name="rcp", bufs=2) as rcpool,
            tc.tile_pool(name="pscore", bufs=2, space="PSUM") as pscore,
            tc.tile_pool(name="pctx", bufs=2, space="PSUM") as pctx,
            tc.tile_pool(name="psum2", bufs=2, space="PSUM") as psums,
        ):
            ct_full = ctfull.tile([128, HPC, S], F16)
            wo_sb = wop.tile([128, HPC, D], F16)

            # Flat (head, qc, pair) pipeline.  Scores run one pair ahead of
            # ctx, crossing qc/head boundaries so the PE never drains at a
            # boundary; per-qc finalization (denominator matmul + recip +
            # normalize) is deferred two pair-units so its inputs are ready.
            units = []
            for h in range(HPC):
                for qc in range(QC):
                    prs = [(4 * qc, 4 * qc + 1), (4 * qc + 2, 4 * qc + 3)]
                    prs += [(t, t + 1) for t in range(0, 4 * qc, 2)]
                    for pi, pair in enumerate(prs):
                        units.append({
                            "h": h, "qc": qc, "pair": pair,
                            "first": pi == 0, "last": pi == len(prs) - 1,
                        })

            qk_tiles = {0: (head0["qt"], head0["kt"])}

            def load_head(h):
                if h in qk_tiles or h >= HPC:
                    return
                qt_sb = qkpre.tile([128, S], F16, tag="qt", name="qt_sb")
                nc.sync.dma_start(out=qt_sb, in_=qt_d[h * 128 : (h + 1) * 128, :])
                kt_sb = qkpre.tile([128, S], F16, tag="kt", name="kt_sb")
                nc.sync.dma_start(out=kt_sb, in_=kt_d[h * 128 : (h + 1) * 128, :])
                qk_tiles[h] = (qt_sb, kt_sb)

            state = {}      # (h, qc) -> dict with psum_c, psum_s, p_acc
            finalizes = []  # (due_iteration, h, qc)

            def scores(u):
                h, qc = u["h"], u["qc"]
                qt_sb, kt_sb = qk_tiles[h]
                pp = pscore.tile([128, 1024], F32, tag="pp")
                offs = []
                for half, kt_i in enumerate(u["pair"]):
                    j = kt_i - 4 * qc
                    off = 128 * j if j > 0 else 0
                    nc.tensor.matmul(
                        pp[:, half * 512 + off : (half + 1) * 512],
                        kt_sb[:, kt_i * 128 : (kt_i + 1) * 128],
                        qt_sb[:, qc * 512 + off : (qc + 1) * 512],
                        start=True,
                        stop=True,
                    )
                    offs.append((kt_i, j, off))
                p_t = ppool.tile([128, 1024], F16, tag="p_t")
                j0 = offs[0][1]
                # the (j2,j3) diagonal pair is valid only from column 256 on
                e0 = 256 if j0 == 2 else 0
                nc.scalar.activation(
                    out=p_t[:, e0:],
                    in_=pp[:, e0:],
                    func=mybir.ActivationFunctionType.Exp,
                    scale=float(SCALE),
                )
                if j0 >= 0:
                    # zero the causally-invalid region of both halves
                    # (including exp'd garbage columns): element (k, half, u)
                    # is valid iff u - k - 128*(j0+half) >= 0
                    nc.gpsimd.affine_select(
                        out=p_t,
                        in_=p_t,
                        pattern=[[-128, 2], [1, 512]],
                        compare_op=mybir.AluOpType.is_ge,
                        fill=0.0,
                        base=-128 * j0,
                        channel_multiplier=-1,
                    )
                u["p_t"] = p_t
                u["offs"] = offs

            def ctx(u):
                h, qc = u["h"], u["qc"]
                if u["first"]:
                    state[(h, qc)] = {
                        "psum_c": pctx.tile([128, 512], F32, name="psum_c"),
                        "psum_s": psums.tile([128, 512], F32, name="psum_s"),
                        "acc_started": False,
                    }
                st_ = state[(h, qc)]
                p_t, offs = u["p_t"], u["offs"]
                diag = offs[0][1] >= 0
                for half, (kt_i, j, off) in enumerate(offs):
                    seg = p_t[:, half * 512 + off : (half + 1) * 512]
                    nc.tensor.matmul(
                        st_["psum_c"][:, off:],
                        v_full[:, kt_i, h * 128 : (h + 1) * 128],
                        seg,
                        start=(u["first"] and half == 0),
                        stop=(u["last"] and half == 1),
                    )
                    if diag:
                        # diagonal tiles: denominator strip matmul on the PE
                        nc.tensor.matmul(
                            st_["psum_s"][:, off:],
                            ones128,
                            seg,
                            start=(u["first"] and half == 0),
                            stop=(qc == 0 and u["last"] and half == 1),
                        )
                    else:
                        # full tiles: fp16 accumulation on the DVE (exact:
                        # one ones-matmul at finalize sums the partitions)
                        full = p_t[:, half * 512 : (half + 1) * 512]
                        if not st_["acc_started"]:
                            st_["p_acc"] = paccp.tile([128, 512], F16,
                                                      tag="pacc", name="p_acc")
                            nc.vector.tensor_copy(out=st_["p_acc"], in_=full)
                            st_["acc_started"] = True
                        else:
                            nc.vector.tensor_tensor(
                                out=st_["p_acc"],
                                in0=st_["p_acc"],
                                in1=full,
                                op=mybir.AluOpType.add,
                            )

            def finalize(h, qc):
                st_ = state.pop((h, qc))
                if st_["acc_started"]:
                    nc.tensor.matmul(
                        st_["psum_s"],
                        ones128,
                        st_["p_acc"],
                        start=False,
                        stop=True,
                    )
                recip = rcpool.tile([128, 512], F32, tag="rcp")
                nc.vector.reciprocal_approx_fast(out=recip, in_=st_["psum_s"])
                nc.vector.tensor_tensor(
                    out=ct_full[:, h, qc * 512 : (qc + 1) * 512],
                    in0=st_["psum_c"],
                    in1=recip,
                    op=mybir.AluOpType.mult,
                )

            for i, u in enumerate(units):
                if u["first"] and u["qc"] == 1:
                    load_head(u["h"] + 1)  # prefetch next head's q/k early
                if u["first"] and u["qc"] == 0:
                    # spread the 4MB Wo load through phase B
                    nc.sync.dma_start(out=wo_sb[:, u["h"], :],
                                      in_=wo_r[:, u["h"], :])
                scores(u)
                while finalizes and finalizes[0][0] <= i:
                    _, fh, fqc = finalizes.pop(0)
                    finalize(fh, fqc)
                if i > 0:
                    pu = units[i - 1]
                    ctx(pu)
                    if pu["last"]:
                        finalizes.append((i + 1, pu["h"], pu["qc"]))
            ctx(units[-1])
            finalizes.append((0, units[-1]["h"], units[-1]["qc"]))
            for _, fh, fqc in finalizes:
                finalize(fh, fqc)

        # ---------------- Phase C: output projection -------------------------
        with (
            tc.tile_pool(name="opsum", bufs=4, space="PSUM") as ops,
            tc.tile_pool(name="ostage", bufs=4) as ost,
        ):
            for st in range(ST):
                for ncol in range(4):
                    psum = ops.tile([128, 512], F32)
                    for hh in range(HPC):
                        nc.tensor.matmul(
                            psum,
                            ct_full[:, hh, st * 128 : (st + 1) * 128],
                            wo_sb[:, hh, ncol * 512 : (ncol + 1) * 512],
                            start=(hh == 0),
                            stop=(hh == HPC - 1),
                        )
                    o_sb = ost.tile([128, 512], F32, tag="ostage")
                    nc.scalar.activation(
                        out=o_sb, in_=psum, func=mybir.ActivationFunctionType.Copy
                    )
                    nc.gpsimd.dma_start(
                        out=out[
                            st * 128 : (st + 1) * 128,
                            ncol * 512 : (ncol + 1) * 512,
                        ],
                        in_=o_sb,
                    )
        wop_cm.__exit__(None, None, None)
        ctfull_cm.__exit__(None, None, None)


_NC = None


def _get_nc():
    global _NC
    if _NC is None:
        _NC = _build_nc()
    return _NC


def _host_prep(input_sequences, Wq, bq, Wk, bk, Wv, bv, Wo, bo):
    """Build per-core input maps."""
    x = np.asarray(input_sequences, dtype=np.float32)

    in_maps = []
    for c in range(8):
        b, g = divmod(c, 2)
        sl = slice(g * DHG, (g + 1) * DHG)
        wq_c = np.ascontiguousarray(
            np.asarray(Wq[:, sl], dtype=np.float32)
            .reshape(KT, 128, HPC, 128).transpose(2, 1, 0, 3).reshape(DHG, D)
        ).astype(np.float16)
        wk_c = np.ascontiguousarray(
            np.asarray(Wk[:, sl], dtype=np.float32)
            .reshape(KT, 128, HPC, 128).transpose(2, 1, 0, 3).reshape(DHG, D)
        ).astype(np.float16)
        wv_c = np.ascontiguousarray(Wv[:, sl]).astype(np.float16)
        wo_c = np.ascontiguousarray(Wo[sl, :]).astype(np.float16)
        in_maps.append({
            "xT": np.ascontiguousarray(x[b].T).astype(np.float16),
            "wq": wq_c,
            "wk": wk_c,
            "wv": wv_c,
            "wo": wo_c,
            "bqT": np.ascontiguousarray(
                np.asarray(bq[sl], dtype=np.float32).reshape(HPC, 128).T
            ),
            "bkT": np.ascontiguousarray(
                np.asarray(bk[sl], dtype=np.float32).reshape(HPC, 128).T
            ),
            "bvb": np.ascontiguousarray(
                np.broadcast_to(np.asarray(bv[sl], dtype=np.float32), (128, DHG))
            ),
        })
    return in_maps


def kernel(input_sequences, Wq, bq, Wk, bk, Wv, bv, Wo, bo, _trace=False):
    nc = _get_nc()
    in_maps = _host_prep(input_sequences, Wq, bq, Wk, bk, Wv, bv, Wo, bo)
    res = run_bass_kernel_spmd(nc, in_maps, list(range(8)), trace=_trace)
    bo32 = np.asarray(bo, dtype=np.float32)
    out = np.empty((B, S, D), dtype=np.float32)
    for b in range(B):
        out[b] = res.results[2 * b]["out"] + res.results[2 * b + 1]["out"] + bo32
    if _trace:
        kernel.last_exec_time_ns = res.exec_time_ns
    return out
